# revision 25
# baseline (speedup 1.0000x reference)
"""Trainium2 Bass kernel for nn_NeuralAESImplementation.

Restructured math (state-major layout, p = 32j + 8i + k  <->  st[b, i, j, k]):
  round-0 ARK folded into round-1 weights (binary input: |x-K0| = (1-2K0)x + K0)
  round rho = 1..9:
    h[j]  = relu(L1D[32j:32j+32,:]^T x[32j:32j+32] + b)   (PE, 4x row-tiled)
    s     = sum_j lhsT2mix[j]^T h[j] + MC^T x  (sub_bytes W1 + shift + mix)
    x'    = |g4(s) - k| at ASTAR scale:        (3-op gadget, k = round-key bit)
            sb = Relu(s*sc1 + c)               [ACT, PSUM->SBUF]
            z  = arw(sb; A(K-2), A, 2A)        [DVE ADD_RANGE_WRAP]
            x' = |z|                           [DVE int16 bitwise_and 0x7fff]
            (alpha_r/ASTAR rescale of x' folded into round-r+1 weights)
  round 10: s = sum_j FIN[j]^T h[j]; out = |fsc*s - K10|  (ACT Abs)

PE: the 4 mm1 matrices have disjoint 32-row contraction supports, stored
summed as one dense 128x128; issued as 4 concurrent K=32 tile_position
matmuls (tile_position=(32j,0)) -> ~1 dense-matmul span on HW instead of 4.
mm2 (4 dense) + MC correction (1 dense) accumulate in one PSUM bank.

Weight f16 truncation corrected by sampled MC matrices (corr=2); FIN's
1/alpha9 scale can overflow f16 -> power-of-2 excess moved to the final
activation scale (wvec col 40).

Data parallel over 8 cores (batch/8 each); host transposes to/from
state-major [128, B]. Batch tiled by 512 (PSUM bank).
"""

import os

import numpy as np

B_TOTAL = 131072
N_CORES = 8
B_CORE = B_TOTAL // N_CORES
NT = 512  # batch tile
SBOX_H = 32
BIG = 1.0e30

_CACHE = {}
_RUN_KWARGS = {}


def _env(name, default):
    return os.environ.get(name, default)


# ---------------------------------------------------------------- host math
def _rtn11(w):
    """Round-to-nearest at 11 explicit mantissa bits (matches HW f32r)."""
    u = np.asarray(w, np.float32).view(np.uint32)
    hi = (((u.astype(np.uint64) + 0x800) & 0xFFFFF000).astype(np.uint32)).view(
        np.float32
    )
    return hi


def _build_mats(w0, w1, K0):
    """lhsT1 (with ARK0 fold variant), lhsT2mix, lhsT2fin."""
    lhsT1 = np.zeros((4, 128, 128), np.float32)
    lhsT1f = np.zeros((4, 128, 128), np.float32)
    lhsT2mix = np.zeros((4, 128, 128), np.float32)
    lhsT2fin = np.zeros((4, 128, 128), np.float32)
    iu = np.arange(32)
    sgn = 1.0 - 2.0 * K0  # [128]
    for j in range(4):
        for i in range(4):
            for k in range(8):
                p = 32 * j + 8 * i + k
                lhsT1[j][p, 32 * i + iu] = w0[:, k]
                lhsT1f[j][p, 32 * i + iu] = w0[:, k] * sgn[p]
            ip = (i - j) % 4
            for jp in range(4):
                d = (j - jp) % 4
                for k in range(8):
                    if d <= 2:
                        lhsT2mix[j][32 * i + iu, 32 * jp + 8 * ip + k] = w1[k, :]
                    if d == 0:
                        lhsT2fin[j][32 * i + iu, 32 * jp + 8 * ip + k] = w1[k, :]
    return lhsT1, lhsT1f, lhsT2mix, lhsT2fin


# wmat layout (95 matrices, fixed):
#   0     L1F_D (round-1 mm1, ARK0-folded, dense sum of 4 disjoint j-blocks)
#   1-9   L1D_r (mm1 round r=2..10, dense sum, scaled alpha_{r-1}/ASTAR)
#   10-13 FIN (round-10 mm2, scaled 1/alpha9)
#   14-49 HI_r (mm2 hi, rounds 1..9, scaled g_r)      [14+4(r-1)+j]
#   50-85 LO_r (mm2 lo residual, rounds 1..9)         [50+4(r-1)+j]
#   86-94 MC_r (truncation-correction, rounds 1..9)   [85+r]
# The per-j mm1 matrices lhsT1[j] have disjoint row supports (rows 32j..32j+31),
# so their sum is a single 128x128 matrix; the kernel issues 4 concurrent
# K=32 row-tiled matmuls (tile_position=(32j,0)) against 32-row slices of it.
# Stored state x for rounds >=2 is at fixed ASTAR scale (gadget |z| output,
# no post-scale op); the alpha_r/ASTAR rescale is folded into L1D_{r+1}/MC_{r+1}.
IDX_L1F, IDX_L1, IDX_FIN, IDX_HI, IDX_LO, IDX_MC = 0, 1, 10, 14, 50, 86
NMAT = 95
# wvec columns (f32):
#   0-3   round-1 relu bias per j (ARK0 fold)
#   4-12  relu bias rounds 2..10  (alpha_{r-1} b0)    [2+r]
#   13-21 sc1_r = ASTAR/alpha_r (sb scale)            [12+r]
#   22-30 sb bias = ASTAR c_r                         [21+r]
#   31-39 sig_r = ASTAR (K_r - 2) (arw shift)         [30+r]
#   40-48 sc2_r = alpha_r/ASTAR (abs scale)           [39+r]
#   49    -K10
NVEC = 50
ASTAR = 2.0 ** -8  # fixed gadget scale: gadget ops see ASTAR * unscaled values


def _rnd_weight(w, dt):
    if dt == "f16":
        return np.asarray(w, np.float32).astype(np.float16).astype(np.float32)
    return _rtn11(w)


def _host_prep(round_keys, w0, b0, w1, state_sample=None):
    corr = int(_env("NEURAES_CORR", "2"))
    m_split = int(_env("NEURAES_M", "0"))
    dt = _env("NEURAES_DT", "f16")
    # K[r][p], p = 32j+8i+k <-> round_keys[r,0,i,j,k]
    K = (
        np.transpose(round_keys[:, 0], (0, 2, 1, 3))
        .reshape(11, 128)
        .astype(np.float32)
    )
    lhsT1, lhsT1f, lhsT2mix, lhsT2fin = _build_mats(w0, w1, K[0])

    # per-j relu bias: round 1 folds ARK0 (b0 + W0 K0_block)
    b0t = np.tile(b0, 4).astype(np.float64)
    b1f = np.zeros((4, 128), np.float64)
    for j in range(4):
        bj = b0t.copy()
        for i in range(4):
            kblk = K[0][32 * j + 8 * i : 32 * j + 8 * i + 8]
            bj[32 * i : 32 * i + 32] += w0.astype(np.float64) @ kblk
        b1f[j] = bj

    # ---- sample forward (fp64): value ranges + relu statistics
    MC = np.zeros((9, 128, 128), np.float32)
    cvec = np.zeros((9, 128), np.float32)
    alpha = np.ones(11, np.float64)  # alpha[r] scales round-r outputs (x_r, s_r)
    d1f = (lhsT1f - _rnd_weight(lhsT1f, dt)).astype(np.float64)
    d1 = (lhsT1 - _rnd_weight(lhsT1, dt)).astype(np.float64)
    d2 = (lhsT2mix - _rnd_weight(lhsT2mix, dt)).astype(np.float64)
    L1f64 = lhsT1f.astype(np.float64)
    L164 = lhsT1.astype(np.float64)
    L2hi64 = _rnd_weight(lhsT2mix, dt).astype(np.float64)
    L2ex64 = lhsT2mix.astype(np.float64)
    if state_sample is not None:
        x = np.abs(state_sample.astype(np.float64) - K[0])  # ARK0, state-major
        for r in range(1, 10):
            L1r = L1f64 if r == 1 else L164
            D1r = d1f if r == 1 else d1
            b_r = b1f if r == 1 else np.stack([b0t] * 4)
            zs, hs = [], []
            for j in range(4):
                z = x @ L1r[j] + b_r[j]
                zs.append(z)
                hs.append(np.maximum(z, 0.0))
            split = r <= m_split
            L2eff = L2ex64 if split else L2hi64
            mc = np.zeros((128, 128), np.float64)
            cv = np.zeros(128, np.float64)
            for j in range(4):
                z, h = zs[j], hs[j]
                var = z.var(axis=0) + 1e-12
                a_fit = ((h * z).mean(0) - h.mean(0) * z.mean(0)) / var
                b_fit = h.mean(0) - a_fit * z.mean(0)
                a_mask = (z > 0).mean(0)
                # mm1 truncation: s_err ~= D1[j] diag(a_mask) L2eff[j] on x
                mc += D1r[j] @ (a_mask[:, None] * L2eff[j])
                if corr >= 2 and not split:
                    # mm2 truncation: s_err ~= L1 diag(a_fit) d2[j] on x + const
                    mc += L1r[j] @ (a_fit[:, None] * d2[j])
                    cv += d2[j].T @ (b_fit + a_fit * b_r[j])
            MC[r - 1] = mc.astype(np.float32)
            cvec[r - 1] = cv.astype(np.float32)
            # advance sample state exactly (fp64)
            s = sum(hs[j] @ L2ex64[j] for j in range(4))
            g4 = (
                np.maximum(s, 0) - 2 * np.maximum(s - 1, 0)
                + 2 * np.maximum(s - 2, 0) - 2 * np.maximum(s - 3, 0)
            )
            x = np.abs(g4 - K[r])
            if dt == "f16":
                hmax = max(np.abs(hs[j]).max() for j in range(4))
                xmax = np.abs(x).max() + 1e-9
                # keep x'_r <= ~1024 and h'_r (= alpha_{r-1} h) <= ~8192
                a = 2.0 ** -max(0.0, np.ceil(np.log2(xmax / 1024.0)))
                ah = 2.0 ** -max(0.0, np.ceil(np.log2(hmax * alpha[r - 1] / 8192.0)))
                alpha[r] = min(a, 1.0)
                if ah < 1.0:
                    alpha[r - 1] *= ah  # rare: tighten previous round for h range
                    alpha[r] = min(alpha[r], alpha[r - 1])

    # ---- assemble scaled weight stack
    wmat = np.zeros((NMAT, 128, 128), np.float64)
    L1D = lhsT1.sum(axis=0)  # disjoint row supports
    wmat[IDX_L1F] = lhsT1f.sum(axis=0)
    for r in range(2, 11):
        # x input of round r is at ASTAR scale; restore alpha_{r-1} h-scale
        wmat[IDX_L1 + (r - 2)] = L1D * (alpha[r - 1] / ASTAR)
    # FIN = lhsT2fin/alpha9 can overflow f16 when alpha9 is tiny; keep the
    # power-of-2 excess in the final activation's scale operand (wvec col 40)
    fin = lhsT2fin / alpha[9]
    fmax = np.abs(fin).max() + 1e-30
    fsc = 2.0 ** max(0.0, np.ceil(np.log2(fmax / 30000.0)))
    wmat[IDX_FIN : IDX_FIN + 4] = fin / fsc
    for r in range(1, 10):
        g = alpha[r] / alpha[r - 1]
        w2s = lhsT2mix.astype(np.float64) * g
        hi = _rnd_weight(w2s.astype(np.float32), dt).astype(np.float64)
        wmat[IDX_HI + 4 * (r - 1) : IDX_HI + 4 * r] = hi
        if r <= m_split:
            wmat[IDX_LO + 4 * (r - 1) : IDX_LO + 4 * r] = w2s - hi
        if corr > 0:
            xs = alpha[r - 1] / ASTAR if r >= 2 else 1.0
            wmat[IDX_MC + (r - 1)] = MC[r - 1] * g * xs

    wvec = np.zeros((128, NVEC), np.float32)
    for j in range(4):
        wvec[:, j] = b1f[j]
    for r in range(2, 11):
        wvec[:, 2 + r] = alpha[r - 1] * b0t
    for r in range(1, 10):
        wvec[:, 12 + r] = ASTAR / alpha[r]
        wvec[:, 21 + r] = ASTAR * cvec[r - 1]
        wvec[:, 30 + r] = ASTAR * (K[r] - 2.0)
        wvec[:, 39 + r] = alpha[r] / ASTAR
    wvec[:, 40] = fsc  # final-round output scale (FIN f16-overflow guard)
    wvec[:, 49] = -K[10]

    np_dt = np.float16 if dt == "f16" else np.float32
    return wmat.astype(np_dt), wvec, alpha


def _fallback_numpy(state, round_keys, xorw, xorb, w0, b0, w1):
    def relu(v):
        return np.maximum(v, 0.0)

    def ark(s, k):
        c0 = xorw[0, 0] * s + xorw[0, 1] * k + xorb[0]
        c1 = xorw[1, 0] * s + xorw[1, 1] * k + xorb[1]
        return relu(c0) + relu(c1)

    def sub_bytes(x):
        h = relu(np.einsum("bijk,hk->bijh", x, w0) + b0)
        return np.einsum("bijh,kh->bijk", h, w1)

    def shift_rows(x):
        return np.stack(
            [np.roll(x[:, :, r, :], -r, axis=1) for r in range(4)], axis=2
        )

    def mix_columns(x):
        s = x + np.roll(x, -1, axis=2) + np.roll(x, -2, axis=2)
        return relu(s) - 2 * relu(s - 1) + 2 * relu(s - 2) - 2 * relu(s - 3)

    st = state.reshape(-1, 4, 4, 8).swapaxes(1, 2)
    st = ark(st, round_keys[0])
    for r in range(1, 10):
        st = mix_columns(shift_rows(sub_bytes(st)))
        st = ark(st, round_keys[r])
    st = shift_rows(sub_bytes(st))
    st = ark(st, round_keys[10])
    return np.ascontiguousarray(st.swapaxes(1, 2).reshape(-1, 128), dtype=np.float32)


# ---------------------------------------------------------------- bass program
def _build_bass(b_core):
    import concourse.bacc as bacc
    import concourse.mybir as mybir
    import concourse.tile as tile
    from contextlib import ExitStack

    from concourse.dve_ops import ADD_RANGE_WRAP

    f32 = mybir.dt.float32
    f32r = mybir.dt.float32r
    f16 = mybir.dt.float16
    alu = mybir.AluOpType
    AF = mybir.ActivationFunctionType
    nchunk = b_core // NT
    m_split = int(_env("NEURAES_M", "0"))  # split mm2 in rounds 1..m_split
    corr = int(_env("NEURAES_CORR", "2"))
    use_f16 = _env("NEURAES_DT", "f16") == "f16"
    dt = f16 if use_f16 else f32
    # ACT/DVE split of the h relu: ACT covers h_sb cols [0, rsplit),
    # DVE [rsplit, 4NT). The gadget is sb(ACT) + arw(DVE) + abs(ABS_ENG).
    rsplit = int(_env("NEURAES_RSPLIT", "1024"))
    assert 0 <= rsplit <= 4 * NT
    abs_eng = _env("NEURAES_ABS", "dve")  # dve | gps | act

    nc = bacc.Bacc()
    st_d = nc.dram_tensor("state", [128, b_core], dt, kind="ExternalInput")
    wm_d = nc.dram_tensor("wmat", [NMAT, 128, 128], dt, kind="ExternalInput")
    wv_d = nc.dram_tensor("wvec", [128, NVEC], f32, kind="ExternalInput")
    out_d = nc.dram_tensor("out", [128, b_core], f32, kind="ExternalOutput")

    with tile.TileContext(nc) as tc, ExitStack() as ctx:
        nstream_pool = min(int(_env("NEURAES_NSTREAM", "32")), nchunk)
        koff0 = int(_env("NEURAES_K", "2"))
        use_pair = (_env("NEURAES_PAIR", "1") == "1"
                    and nstream_pool % 2 == 0 and koff0 % 2 == 0)
        xb_def = (nstream_pool // 2 + 2) if use_pair else (nstream_pool + 2)
        wpool = ctx.enter_context(tc.tile_pool(name="weights", bufs=1))
        iopool = ctx.enter_context(tc.tile_pool(name="io", bufs=6))
        xpool = ctx.enter_context(
            tc.tile_pool(name="x", bufs=int(_env("NEURAES_XB", str(xb_def))))
        )
        hpool = ctx.enter_context(
            tc.tile_pool(name="h", bufs=int(_env("NEURAES_HB", "4")))
        )
        gpool = ctx.enter_context(
            tc.tile_pool(
                name="gad",
                bufs=int(_env("NEURAES_GB", "6" if use_pair else "12")),
            )
        )
        fpool = ctx.enter_context(tc.tile_pool(name="f", bufs=4))
        ps_h = ctx.enter_context(
            tc.tile_pool(name="ph", bufs=int(_env("NEURAES_PHB", "3")), space="PSUM")
        )
        ps_s = ctx.enter_context(
            tc.tile_pool(
                name="ps",
                bufs=int(_env("NEURAES_PSB", "1" if use_pair else "2")),
                space="PSUM",
            )
        )

        wsb = wpool.tile([128, NMAT * 128], dt, tag="wsb")
        if use_f16:
            nc.sync.dma_start(
                wsb[:].rearrange("p (m q) -> p m q", m=NMAT),
                wm_d[:].rearrange("m p q -> p m q"),
            )
        else:
            nc.sync.dma_start(
                wsb[:].rearrange("p (m q) -> p m q", m=NMAT).bitcast(f32r),
                wm_d[:].rearrange("m p q -> p m q").bitcast(f32r),
            )
        vec_sb = wpool.tile([128, NVEC], f32, tag="vec")
        nc.sync.dma_start(vec_sb[:], wv_d[:])

        def W(m):
            sl = wsb[:, 128 * m : 128 * (m + 1)]
            return sl if use_f16 else sl.bitcast(f32r)

        def mmcast(ap):
            return ap if use_f16 else ap.bitcast(f32r)

        def V(c):
            return vec_sb[:, c : c + 1]

        nstream = min(int(_env("NEURAES_NSTREAM", "32")), nchunk)
        koff = int(_env("NEURAES_K", "2"))
        assert nchunk % nstream == 0

        def emit_mm1_relu(c, rho, st):
            first = rho == 1
            if first:
                x_t = iopool.tile([128, NT], dt, tag="in")
                nc.sync.dma_start(
                    mmcast(x_t[:]), mmcast(st_d[:, c * NT : (c + 1) * NT])
                )
                x_sb = x_t[:]  # AP; rounds >=2 store an AP slice directly
            else:
                x_sb = st["x"]

            # --- mm1: h[j] = L1D[32j:32j+32,:]^T x[32j:32j+32]  (4 row-tiled
            # K=32 matmuls on disjoint 32-row sub-arrays; they run
            # concurrently on the PE at ~1 dense-matmul span total)
            base1 = IDX_L1F if first else IDX_L1 + (rho - 2)
            h_ps = []
            for half in range(2):
                hp = ps_h.tile([128, 2 * NT], f32, tag="h")
                for jj in range(2):
                    j = 2 * half + jj
                    nc.tensor.matmul(
                        hp[:, NT * jj : NT * (jj + 1)],
                        W(base1)[32 * j : 32 * (j + 1), :],
                        mmcast(x_sb[32 * j : 32 * (j + 1), :]),
                        start=True,
                        stop=True,
                        skip_group_check=True,
                        tile_position=(32 * j, 0),
                    )
                h_ps.append(hp)

            # --- relu + bias -> h_sb: ACT covers [0, rsplit), DVE the rest,
            # with instructions clipped to psum-tile and (round-1) j-block
            # boundaries.
            h_sb = hpool.tile([128, 4 * NT], dt, tag="hs")

            def relu_range(lo, hi, eng):
                if hi <= lo:
                    return
                bcol = (lo // NT) if first else (rho + 2)
                t = 0 if lo < 2 * NT else 1
                plo, phi = lo - 2 * NT * t, hi - 2 * NT * t
                if eng == "A":
                    nc.scalar.activation(h_sb[:, lo:hi], h_ps[t][:, plo:phi],
                                         AF.Relu, bias=V(bcol))
                else:
                    nc.vector.tensor_scalar(h_sb[:, lo:hi], h_ps[t][:, plo:phi],
                                            V(bcol), 0.0, alu.add, alu.max)

            # boundaries: engine split + psum tile edge (+ j blocks in round 1)
            cuts = {0, rsplit, 2 * NT, 4 * NT}
            if first:
                cuts |= {NT, 3 * NT}
            cuts = sorted(cuts)
            for lo, hi in zip(cuts[:-1], cuts[1:]):
                relu_range(lo, hi, "A" if lo < rsplit else "D")
            st["h"] = h_sb
            st["xin"] = x_sb

        def emit_mm2_gadget(c, rho, st, c2=None, st2=None):
            # paired form: chunks (c, c2) share one 2-bank s tile so the
            # gadget/output ops run once at FD=2*NT (halves per-op inits
            # and real-HW drain count)
            pair = st2 is not None
            W_OUT = 2 * NT if pair else NT
            s_ps = ps_s.tile([128, W_OUT], f32, tag="s")
            do_split = rho < 10 and rho <= m_split
            # MCDROP=k drops the MC correction matmul for the last k gadget
            # rounds (smallest downstream amplification); cvec stays applied.
            mcdrop = int(_env("NEURAES_MCDROP", "0"))
            do_corr = corr > 0 and rho < 10 and rho <= 9 - mcdrop
            nmm = (8 if do_split else 4) + (1 if do_corr else 0)
            bases = ([IDX_FIN] if rho == 10 else
                     [IDX_HI + 4 * (rho - 1)]
                     + ([IDX_LO + 4 * (rho - 1)] if do_split else []))
            # matrix-major over the pair: each stationary weight is used by
            # both chunks back-to-back (halves effective LDWEIGHTS traffic);
            # the two chunks accumulate into different PSUM banks so the
            # interleaved start/stop groups are independent.
            sts = (st, st2) if pair else (st,)
            hs = [stk.pop("h") for stk in sts]
            xs = [stk.pop("xin") for stk in sts]
            ops = ([(IDX_MC + rho - 1, "x")] if do_corr else [])
            for b in bases:
                ops += [(b + j, j) for j in range(4)]
            assert len(ops) == nmm
            for oi, (m, src) in enumerate(ops):
                for k in range(len(sts)):
                    rhs = (xs[k] if src == "x"
                           else hs[k][:, NT * src : NT * (src + 1)])
                    nc.tensor.matmul(
                        s_ps[:, k * NT : k * NT + NT], W(m), mmcast(rhs),
                        start=(oi == 0), stop=(oi == nmm - 1),
                        skip_group_check=True,
                    )

            if rho < 10:
                # --- gadget: x = dist(relu(s)+c, {0,2,4} or {1,3}) at the
                # fixed ASTAR scale (one op set per pair):
                #   sb = Relu(ASTAR/alpha_r * s + ASTAR*c_r)       [ACT]
                #   z  = arw(sb; ASTAR*(K-2), ASTAR, 2*ASTAR)      [DVE]
                #   x  = |z|  (int16 bitwise_and sign clear)       [DVE 4x]
                sb = gpool.tile([128, W_OUT], dt, tag="sb")
                nc.scalar.activation(sb[:], s_ps[:], AF.Relu,
                                     bias=V(21 + rho), scale=V(12 + rho))
                z = gpool.tile([128, W_OUT], dt, tag="z")
                nc.vector._custom_dve(
                    ADD_RANGE_WRAP, out=z[:], in0=sb[:], s0=V(30 + rho),
                    s1=ASTAR, imm2=2.0 * ASTAR,
                )
                x_sb = xpool.tile([128, W_OUT], dt, tag="x")
                i16 = mybir.dt.int16
                if abs_eng == "act":
                    nc.scalar.activation(x_sb[:], z[:], AF.Abs)
                else:
                    nc.vector.tensor_scalar(
                        x_sb[:].bitcast(i16), z[:].bitcast(i16), 0x7FFF, None,
                        alu.bitwise_and,
                    )
                st["x"] = x_sb[:, 0:NT]
                if pair:
                    st2["x"] = x_sb[:, NT : 2 * NT]
            else:
                f_sb = fpool.tile([128, W_OUT], f32, tag="fout")
                nc.scalar.activation(f_sb[:], s_ps[:], AF.Abs, bias=V(49),
                                     scale=V(40))
                nc.sync.dma_start(out_d[:, c * NT : c * NT + W_OUT], f_sb[:])

        # Software-pipelined emission: mm2+gadget of stream i-koff is emitted
        # between mm1 of streams i and i+1, so the PE always has independent
        # matmul work during the relu/gadget latency of any one stream.
        # With pairing, mm2+gadget is emitted once per CONSECUTIVE chunk pair.
        for grp in range(nchunk // nstream):
            chunks = [grp * nstream + k for k in range(nstream)]
            states = [dict() for _ in chunks]
            for rho in range(1, 11):
                for i, (c, st) in enumerate(zip(chunks, states)):
                    emit_mm1_relu(c, rho, st)
                    if i < koff:
                        continue
                    p = i - koff
                    if use_pair:
                        if p % 2 == 1:
                            emit_mm2_gadget(chunks[p - 1], rho, states[p - 1],
                                            chunks[p], states[p])
                    else:
                        emit_mm2_gadget(chunks[p], rho, states[p])
                tail = range(nstream - koff, nstream)
                if use_pair:
                    for i in tail[::2]:
                        emit_mm2_gadget(chunks[i], rho, states[i],
                                        chunks[i + 1], states[i + 1])
                else:
                    for i in tail:
                        emit_mm2_gadget(chunks[i], rho, states[i])

    nc.compile()
    return nc


def _get_bass(b_core):
    key = ("nc", b_core) + tuple(
        sorted((k, v) for k, v in os.environ.items() if k.startswith("NEURAES_"))
    )
    if key not in _CACHE:
        _CACHE[key] = _build_bass(b_core)
    return _CACHE[key]


# ---------------------------------------------------------------- entry point
def kernel(**inputs):
    state = np.ascontiguousarray(np.asarray(inputs["state_tensor"], np.float32))
    rk = np.asarray(inputs["round_keys"], np.float32)
    xorw = np.asarray(inputs["xorw"], np.float32)
    xorb = np.asarray(inputs["xorb"], np.float32)
    w0 = np.asarray(inputs["sbox_w0"], np.float32)
    b0 = np.asarray(inputs["sbox_b0"], np.float32)
    w1 = np.asarray(inputs["sbox_w1"], np.float32)

    canonical = (
        np.array_equal(xorw, np.array([[1.0, -1.0], [-1.0, 1.0]], np.float32))
        and np.all(xorb == 0.0)
        and np.all((rk == 0.0) | (rk == 1.0))
        and np.all((state == 0.0) | (state == 1.0))
        and state.shape == (B_TOTAL, 128)
        and w0.shape == (SBOX_H, 8)
        and w1.shape == (8, SBOX_H)
    )
    if not canonical:
        return _fallback_numpy(state, rk, xorw, xorb, w0, b0, w1)

    wmat, wvec, _alpha = _host_prep(rk, w0, b0, w1, state_sample=state[:2048])
    np_dt = np.float16 if _env("NEURAES_DT", "f16") == "f16" else np.float32
    state_t = np.ascontiguousarray(state.T.astype(np_dt))  # [128, B]

    nc = _get_bass(B_CORE)
    from concourse.bass_utils import run_bass_kernel_spmd

    in_maps = []
    for c in range(N_CORES):
        in_maps.append(
            {
                "state": np.ascontiguousarray(
                    state_t[:, c * B_CORE : (c + 1) * B_CORE]
                ),
                "wmat": wmat,
                "wvec": wvec,
            }
        )
    res = run_bass_kernel_spmd(nc, in_maps, list(range(N_CORES)), **_RUN_KWARGS)
    _CACHE["last_result"] = res
    out_t = np.concatenate([res.results[c]["out"] for c in range(N_CORES)], axis=1)
    return np.ascontiguousarray(out_t.T, np.float32)

